# revision 36
# baseline (speedup 1.0000x reference)
"""HSTU-style dense transformer for sequence modeling on 8 Trainium2 NeuronCores.

Sharding: data-parallel over batch (B=8 -> 1 sequence per core). All weights
replicated. Activations are kept feature-major [D=partitions, T=free] on chip;
GEMMs consume bf16 weights as stationary lhsT tiles (fast weight load) and
bf16 activations as the moving operand; attention computes transposed scores
[kt, qt] so the sigmoid/causal-trim/AV chain needs no on-device transposes.

Per-layer critical path: the V projection consumes RAW x (LayerNorm's
per-token affine commutes with the GEMM): v = (x@Wv)*rs[t] - mu[t]*rs[t]*
colsum(Wv), applied via a K=1 rank-1 finisher matmul plus a per-partition
scale at PSUM drain. This keeps the PE busy through the serial LN row-math
chain (which otherwise idles the PE long enough for the HAM clock gate to
drop it to 1.2 GHz).

Host side only marshals: embedding gather + positional add, weight pre-tiling
into DMA-contiguous bf16 layouts, and the final [V,T] -> [S,V] untranspose.
"""

import sys

sys.path.insert(0, "/opt/trn_rl_repo")

import numpy as np

import concourse.bass as bass  # noqa: F401  (keeps bass registered before bacc)
import concourse.tile as tile
from concourse import bacc, mybir
from concourse.bass import ts
from concourse.bass_utils import run_bass_kernel_spmd

B, S, D, H, L, V = 8, 512, 1024, 16, 6, 32000
DH = D // H
LN_EPS = 1e-5
N_CORES = 8
NC_D = D // 128      # 8 feature chunks
NC_T = S // 128      # 4 token chunks
NC_V = V // 128      # 250 vocab chunks
NP = 8               # head pairs

F32 = mybir.dt.float32
BF16 = mybir.dt.bfloat16
AF = mybir.ActivationFunctionType
OP = mybir.AluOpType

_prog_cache = {}


def _build(cfg):
    """Build + compile the SPMD per-core program. cfg is a hashable tuple."""
    (mm_mode, use_lng, use_lnb, use_bqk, use_bv, use_bg, use_bo, use_bp, rpb_nz) = cfg
    assert mm_mode == "bf16"
    DT = BF16

    nc = bacc.Bacc("TRN2", target_bir_lowering=False, debug=False)

    x0_d = nc.dram_tensor("x0t", [NC_D, 128, S], DT, kind="ExternalInput").ap()
    wqk_d = nc.dram_tensor("wqk", [L, 16, 128, 1024], DT, kind="ExternalInput").ap()
    wv_d = nc.dram_tensor("wv", [L, 16, 128, 512], DT, kind="ExternalInput").ap()
    wvs_d = nc.dram_tensor("wvs", [L, 2, 1, 512], DT, kind="ExternalInput").ap()
    wvb_d = nc.dram_tensor("wvb", [L, 2, 1, 512], DT, kind="ExternalInput").ap() if use_lnb else None
    wg_d = nc.dram_tensor("wg", [L, NC_D, 128, 1024], DT, kind="ExternalInput").ap()
    wo_d = nc.dram_tensor("wo", [L, NC_D, 128, 1024], DT, kind="ExternalInput").ap()
    wp_d = nc.dram_tensor("wp", [NC_V, 128, 1024], DT, kind="ExternalInput").ap()
    tri_d = nc.dram_tensor("tri", [128, 128], DT, kind="ExternalInput").ap()
    onec_d = nc.dram_tensor("onec", [128, 1], DT, kind="ExternalInput").ap()
    oner_d = nc.dram_tensor("oner", [1, 128], DT, kind="ExternalInput").ap()
    lng_d = nc.dram_tensor("lng", [L, NC_D, 128], F32, kind="ExternalInput").ap() if use_lng else None
    lnb_d = nc.dram_tensor("lnb", [L, NC_D, 128], F32, kind="ExternalInput").ap() if use_lnb else None
    bqk_d = nc.dram_tensor("bqk", [L, 16, 128], F32, kind="ExternalInput").ap() if use_bqk else None
    bv_d = nc.dram_tensor("bv", [L, 2, 1, 512], DT, kind="ExternalInput").ap() if use_bv else None
    bg_d = nc.dram_tensor("bg", [L, NC_D, 128], F32, kind="ExternalInput").ap() if use_bg else None
    bo_d = nc.dram_tensor("bo", [L, NC_D, 128], F32, kind="ExternalInput").ap() if use_bo else None
    bp_d = nc.dram_tensor("bp", [NC_V, 128], F32, kind="ExternalInput").ap() if use_bp else None
    rpb_d = nc.dram_tensor("rpb", [1, H * L], DT, kind="ExternalInput").ap() if rpb_nz else None
    out_d = nc.dram_tensor("logits_t", [NC_V, 128, S], DT, kind="ExternalOutput").ap()

    with tile.TileContext(nc) as tc, nc.allow_low_precision(
        reason="bf16 tiles feed the PE; accumulation stays fp32 in PSUM"
    ):
        from contextlib import ExitStack

        with ExitStack() as ctx:
            cp = ctx.enter_context(tc.tile_pool(name="consts", bufs=1))
            xp = ctx.enter_context(tc.tile_pool(name="x", bufs=2))
            xnp = ctx.enter_context(tc.tile_pool(name="xn", bufs=1))
            up = ctx.enter_context(tc.tile_pool(name="u", bufs=1))
            vp = ctx.enter_context(tc.tile_pool(name="v", bufs=1))
            gp = ctx.enter_context(tc.tile_pool(name="g", bufs=1))
            qkp = ctx.enter_context(tc.tile_pool(name="qk", bufs=3))
            tmp = ctx.enter_context(tc.tile_pool(name="tmp", bufs=3))
            rows = ctx.enter_context(tc.tile_pool(name="rows", bufs=6))
            bp_pool = ctx.enter_context(tc.tile_pool(name="bcast", bufs=4))
            atp = ctx.enter_context(tc.tile_pool(name="at", bufs=6))
            wbp = ctx.enter_context(tc.tile_pool(name="wb", bufs=6))
            wvp = ctx.enter_context(tc.tile_pool(name="wvp", bufs=1))
            op_pool = ctx.enter_context(tc.tile_pool(name="out", bufs=4))
            prm = ctx.enter_context(tc.tile_pool(name="prm", bufs=2))
            rcp = ctx.enter_context(tc.tile_pool(name="rc", bufs=2))
            pmm = ctx.enter_context(tc.tile_pool(name="pmm", bufs=2, space="PSUM"))
            pao = ctx.enter_context(tc.tile_pool(name="pao", bufs=2, space="PSUM"))
            psc = ctx.enter_context(tc.tile_pool(name="psc", bufs=3, space="PSUM"))
            pst = ctx.enter_context(tc.tile_pool(name="pst", bufs=1, space="PSUM"))

            mm = nc.tensor.matmul

            onec = cp.tile([128, 1], DT)
            nc.sync.dma_start(onec[:], onec_d[:])
            oner = cp.tile([1, 128], DT)
            nc.sync.dma_start(oner[:], oner_d[:])

            x_cur = xp.tile([128, NC_D * S], DT, tag="x")
            for c in range(NC_D):
                eng = nc.sync if c % 2 == 0 else nc.gpsimd
                eng.dma_start(x_cur[:, ts(c, S)], x0_d[c])

            tri_t = cp.tile([128, 128], DT)
            nc.sync.dma_start(tri_t[:], tri_d[:])
            if rpb_nz:
                rpb_row = cp.tile([1, H * L], DT)
                nc.sync.dma_start(rpb_row[:], rpb_d[:])
                # broadcast to [128, H*L] so column slices give per-partition bias
                prb = psc.tile([128, 512], F32, tag="sc")
                mm(prb[:, : H * L], oner[:], rpb_row[:], start=True, stop=True)
                rpb_t = cp.tile([128, H * L], F32)
                nc.scalar.copy(rpb_t[:], prb[:, : H * L])
            if use_bp:
                bp_t = cp.tile([128, NC_V], F32)
                nc.sync.dma_start(bp_t[:], bp_d.rearrange("v p -> p v"))

            for l in range(L):
                # ---- per-layer params + weight prefetch ----
                wv_t = wvp.tile([128, 16 * 512], DT, tag="wv")
                for i in range(16):
                    nc.gpsimd.dma_start(wv_t[:, ts(i, 512)], wv_d[l, i])
                wvs_t = prm.tile([1, 1024], DT, tag="wvs")
                nc.gpsimd.dma_start(wvs_t[:, 0:512], wvs_d[l, 0])
                nc.gpsimd.dma_start(wvs_t[:, 512:1024], wvs_d[l, 1])
                if use_lng:
                    lng_t = prm.tile([128, NC_D], F32, tag="lng")
                    nc.sync.dma_start(lng_t[:], lng_d[l].rearrange("c p -> p c"))
                if use_lnb:
                    lnb_t = prm.tile([128, NC_D], F32, tag="lnb")
                    nc.sync.dma_start(lnb_t[:], lnb_d[l].rearrange("c p -> p c"))
                if use_bqk:
                    bqk_t = prm.tile([128, 16], F32, tag="bqk")
                    nc.sync.dma_start(bqk_t[:], bqk_d[l].rearrange("c p -> p c"))
                if use_bg:
                    bg_t = prm.tile([128, NC_D], F32, tag="bg")
                    nc.sync.dma_start(bg_t[:], bg_d[l].rearrange("c p -> p c"))
                if use_bo:
                    bo_t = prm.tile([128, NC_D], F32, tag="bo")
                    nc.sync.dma_start(bo_t[:], bo_d[l].rearrange("c p -> p c"))

                # ---- LayerNorm stats (feature-major: reduce over partitions via
                # ones-matmul). Sum lands on psum partition 0, sum-of-squares on
                # partition 32 (32-aligned row groups) -> one PSUM bank total.
                ps = pst.tile([64, S], F32, tag="st")
                for c in range(NC_D):
                    xc = x_cur[:, ts(c, S)]
                    mm(ps[0:1, :], onec[:], xc, start=(c == 0), stop=(c == NC_D - 1))
                    sq = tmp.tile([128, S], DT, tag="tmp")
                    nc.vector.tensor_mul(sq[:], xc, xc)
                    mm(ps[32:33, :], onec[:], sq[:], start=(c == 0), stop=(c == NC_D - 1))

                mu = rows.tile([1, S], F32, tag="row")
                nc.scalar.activation(mu[:], ps[0:1, :], AF.Copy, bias=0.0, scale=1.0 / D)
                nmu16 = rows.tile([1, S], DT, tag="row16")
                nc.scalar.activation(nmu16[:], mu[:], AF.Copy, bias=0.0, scale=-1.0)
                musq = rows.tile([1, S], F32, tag="row")
                nc.vector.tensor_mul(musq[:], mu[:], mu[:])
                var = rows.tile([1, S], F32, tag="row")
                nc.vector.scalar_tensor_tensor(
                    var[:], ps[32:33, :], 1.0 / D, musq[:], op0=OP.mult, op1=OP.subtract
                )
                vare = rows.tile([1, S], F32, tag="row")
                nc.vector.tensor_scalar_add(vare[:], var[:], LN_EPS)
                sd = rows.tile([1, S], F32, tag="row")
                nc.scalar.activation(sd[:], vare[:], AF.Sqrt, bias=0.0, scale=1.0)
                rs = rows.tile([1, S], F32, tag="row")
                nc.vector.reciprocal_approx_fast(rs[:], sd[:])
                msr = rows.tile([1, S], F32, tag="row")
                nc.vector.tensor_mul(msr[:], mu[:], rs[:])

                # rs transposed to column form [128 tok, NC_T] for V-psum drains
                rs_col = rcp.tile([128, NC_T], F32, tag="rsc")
                for c in range(NC_T):
                    nc.sync.dma_start(rs_col[:, c : c + 1], rs[0:1, ts(c, 128)])

                # hi/lo split bf16 broadcasts of rs and mu*rs (fp32 accuracy)
                rs_h = rows.tile([1, S], DT, tag="row16")
                nc.scalar.activation(rs_h[:], rs[:], AF.Copy, bias=0.0, scale=1.0)
                rs_l = rows.tile([1, S], DT, tag="row16")
                nc.vector.tensor_sub(rs_l[:], rs[:], rs_h[:])
                ms_h = rows.tile([1, S], DT, tag="row16")
                nc.scalar.activation(ms_h[:], msr[:], AF.Copy, bias=0.0, scale=1.0)
                ms_l = rows.tile([1, S], DT, tag="row16")
                nc.vector.tensor_sub(ms_l[:], msr[:], ms_h[:])
                pb1 = psc.tile([128, S], F32, tag="sc")
                mm(pb1[:], oner[:], rs_h[:], start=True, stop=False)
                mm(pb1[:], oner[:], rs_l[:], start=False, stop=True)
                pb2 = psc.tile([128, S], F32, tag="sc")
                mm(pb2[:], oner[:], ms_h[:], start=True, stop=False)
                mm(pb2[:], oner[:], ms_l[:], start=False, stop=True)
                rs_b = bp_pool.tile([128, S], F32, tag="bb")
                nc.scalar.copy(rs_b[:], pb1[:])
                ms_b = bp_pool.tile([128, S], F32, tag="bb")
                nc.scalar.copy(ms_b[:], pb2[:])

                # ---- V projection from RAW x, token-major [t, fo].
                # v = ((x - mu) * rs) @ Wv  ==  ((x@Wv) - mu (x) colsum(Wv)) * rs[t]
                # (ln_g is folded into Wv rows host-side; the ln_b/qkv_b terms
                # are token-constant rows, broadcast and added after the rs
                # scaling below.)
                post_add = None
                if use_bv or use_lnb:
                    post_add = []
                    for foB in range(2):
                        pvb = psc.tile([128, 512], F32, tag="sc")
                        first = True
                        if use_bv:
                            bvrow = rows.tile([1, 512], DT, tag="bvr")
                            nc.sync.dma_start(bvrow[:], bv_d[l, foB])
                            mm(pvb[:], oner[:], bvrow[:], start=True, stop=(not use_lnb))
                            first = False
                        if use_lnb:
                            lbrow = rows.tile([1, 512], DT, tag="bvr")
                            nc.sync.dma_start(lbrow[:], wvb_d[l, foB])
                            mm(pvb[:], oner[:], lbrow[:], start=first, stop=True)
                        vbt = bp_pool.tile([128, 512], F32, tag="vb")
                        nc.scalar.copy(vbt[:], pvb[:])
                        post_add.append(vbt)
                v = vp.tile([128, NC_T * 1024], DT, tag="v")
                for foB in range(2):
                    for tc_ in range(NC_T):
                        pv = pmm.tile([128, 512], F32, tag="mm")
                        for fi in range(NC_D):
                            mm(
                                pv[:],
                                x_cur[:, fi * S + tc_ * 128 : fi * S + tc_ * 128 + 128],
                                wv_t[:, ts(foB * 8 + fi, 512)],
                                start=(fi == 0),
                                stop=False,
                            )
                        mm(
                            pv[:],
                            nmu16[0:1, ts(tc_, 128)],
                            wvs_t[0:1, ts(foB, 512)],
                            start=False,
                            stop=True,
                        )
                        dst = v[:, tc_ * 1024 + foB * 512 : tc_ * 1024 + foB * 512 + 512]
                        if post_add is not None:
                            t4 = tmp.tile([128, 512], F32, tag="tmp")
                            nc.vector.tensor_scalar_mul(t4[:], pv[:], rs_col[:, tc_ : tc_ + 1])
                            nc.vector.tensor_add(dst, t4[:], post_add[foB][:])
                        else:
                            nc.vector.tensor_scalar_mul(dst, pv[:], rs_col[:, tc_ : tc_ + 1])

                # ---- xn = (x - mu) * rs, feature-major bf16 ----
                xn = xnp.tile([128, NC_D * S], DT, tag="xn")
                for c in range(NC_D):
                    t = tmp.tile([128, S], F32, tag="tmp")
                    nc.vector.tensor_mul(t[:], x_cur[:, ts(c, S)], rs_b[:])
                    if use_lng or use_lnb:
                        t2 = tmp.tile([128, S], F32, tag="tmp")
                        nc.vector.tensor_sub(t2[:], t[:], ms_b[:])
                        nc.scalar.activation(
                            xn[:, ts(c, S)], t2[:], AF.Identity,
                            bias=(lnb_t[:, c : c + 1] if use_lnb else 0.0),
                            scale=(lng_t[:, c : c + 1] if use_lng else 1.0),
                        )
                    else:
                        nc.vector.tensor_sub(xn[:, ts(c, S)], t[:], ms_b[:])

                # ---- gate: u = silu(xn @ gate_w + bg) (feature-major) ----
                u = up.tile([128, NC_D * S], DT, tag="u")
                for f in range(NC_D):
                    wt = wbp.tile([128, 1024], DT, tag="wb")
                    nc.sync.dma_start(wt[:], wg_d[l, f])
                    pu = pmm.tile([128, S], F32, tag="mm")
                    for fi in range(NC_D):
                        mm(pu[:], wt[:, ts(fi, 128)], xn[:, ts(fi, S)],
                           start=(fi == 0), stop=(fi == NC_D - 1))
                    nc.scalar.activation(
                        u[:, ts(f, S)], pu[:], AF.Silu,
                        bias=(bg_t[:, f : f + 1] if use_bg else 0.0), scale=1.0,
                    )

                # ---- attention, one head pair (= one q/k feature chunk) at a time ----
                g = gp.tile([128, NC_D * S], DT, tag="g")
                for p_ in range(NP):
                    qc = qkp.tile([128, S], DT, tag="qc")
                    kc = qkp.tile([128, S], DT, tag="kc")
                    for (dst, fidx) in ((qc, p_), (kc, 8 + p_)):
                        wt = wbp.tile([128, 1024], DT, tag="wb")
                        nc.sync.dma_start(wt[:], wqk_d[l, fidx])
                        pq = pmm.tile([128, S], F32, tag="mm")
                        for fi in range(NC_D):
                            mm(pq[:], wt[:, ts(fi, 128)], xn[:, ts(fi, S)],
                               start=(fi == 0), stop=(fi == NC_D - 1))
                        if use_bqk:
                            nc.scalar.activation(
                                dst[:], pq[:], AF.Identity,
                                bias=bqk_t[:, fidx : fidx + 1], scale=1.0,
                            )
                        elif dst is kc:
                            # drain the first key chunk separately: score MM
                            # c=0 only needs kc[:, 0:128], so it can issue
                            # before the rest of the drain completes
                            nc.vector.tensor_copy(dst[:, 0:128], pq[:, 0:128])
                            nc.vector.tensor_copy(dst[:, 128:S], pq[:, 128:S])
                        else:
                            nc.vector.tensor_copy(dst[:], pq[:])

                    ao = pao.tile([128, S], F32, tag="ao")
                    for c in range(NC_T):
                        n = S - 128 * c
                        for (o, hh) in ((0, 2 * p_), (64, 2 * p_ + 1)):
                            sc = psc.tile([128, S], F32, tag="sc")
                            mm(sc[:, :n], kc[o : o + 64, ts(c, 128)],
                               qc[o : o + 64, c * 128 : S], start=True, stop=True)
                            sig_bias = rpb_t[:, l * H + hh : l * H + hh + 1] if rpb_nz else 0.0
                            att = atp.tile([128, S], DT, tag="at")
                            nc.scalar.activation(att[:, 0:n], sc[:, 0:n], AF.Sigmoid,
                                                 bias=sig_bias, scale=DH**-0.5)
                            nc.vector.tensor_mul(att[:, 0:128], att[:, 0:128], tri_t[:])
                            mm(
                                ao[o : o + 64, c * 128 : S],
                                v[:, c * 1024 + p_ * 128 + o : c * 1024 + p_ * 128 + o + 64],
                                att[:, 0:n],
                                start=(c == 0),
                                stop=(c == NC_T - 1),
                            )
                    nc.vector.tensor_mul(g[:, ts(p_, S)], ao[:], u[:, ts(p_, S)])

                # ---- out projection + residual ----
                x_new = xp.tile([128, NC_D * S], DT, tag="x")
                for f in range(NC_D):
                    wt = wbp.tile([128, 1024], DT, tag="wb")
                    nc.sync.dma_start(wt[:], wo_d[l, f])
                    pd = pmm.tile([128, S], F32, tag="mm")
                    for fi in range(NC_D):
                        mm(pd[:], wt[:, ts(fi, 128)], g[:, ts(fi, S)],
                           start=(fi == 0), stop=(fi == NC_D - 1))
                    if use_bo:
                        t3 = tmp.tile([128, S], F32, tag="tmp")
                        nc.scalar.activation(t3[:], pd[:], AF.Identity,
                                             bias=bo_t[:, f : f + 1], scale=1.0)
                        nc.vector.tensor_add(x_new[:, ts(f, S)], t3[:], x_cur[:, ts(f, S)])
                    else:
                        nc.vector.tensor_add(x_new[:, ts(f, S)], pd[:], x_cur[:, ts(f, S)])
                x_cur = x_new

            # ---- logits: [V, T] feature(vocab)-major ----
            for vo in range(NC_V):
                wt = wbp.tile([128, 1024], DT, tag="wb")
                nc.sync.dma_start(wt[:], wp_d[vo])
                pl = pmm.tile([128, S], F32, tag="mm")
                for fi in range(NC_D):
                    mm(pl[:], wt[:, ts(fi, 128)], x_cur[:, ts(fi, S)],
                       start=(fi == 0), stop=(fi == NC_D - 1))
                ot = op_pool.tile([128, S], DT, tag="o")
                if use_bp:
                    nc.scalar.activation(ot[:], pl[:], AF.Identity,
                                         bias=bp_t[:, vo : vo + 1], scale=1.0)
                elif vo % 2 == 0:
                    nc.scalar.copy(ot[:], pl[:])
                else:
                    nc.vector.tensor_copy(ot[:], pl[:])
                nc.gpsimd.dma_start(out_d[vo], ot[:])

    nc.compile()
    return nc


def _get_program(cfg):
    nc = _prog_cache.get(cfg)
    if nc is None:
        nc = _build(cfg)
        _prog_cache[cfg] = nc
    return nc


def _marshal(inputs, mm_mode="bf16"):
    """Host-side input marshalling into DMA-friendly layouts."""
    import ml_dtypes

    dt_np = ml_dtypes.bfloat16
    f = np.float32
    input_ids = np.asarray(inputs["input_ids"])
    emb = np.asarray(inputs["embedding"], f)
    pos = np.asarray(inputs["pos_encoding"], f)
    qkv_w = np.asarray(inputs["qkv_w"], f)
    gate_w = np.asarray(inputs["gate_w"], f)
    out_w = np.asarray(inputs["out_w"], f)
    proj_w = np.asarray(inputs["proj_w"], f)

    qkv_b = np.asarray(inputs["qkv_b"], f)
    gate_b = np.asarray(inputs["gate_b"], f)
    out_b = np.asarray(inputs["out_b"], f)
    proj_b = np.asarray(inputs["proj_b"], f)
    ln_g = np.asarray(inputs["ln_g"], f)
    ln_b = np.asarray(inputs["ln_b"], f)
    rpb = np.asarray(inputs["rel_pos_bias"], f)

    use_lng = not np.all(ln_g == 1.0)
    use_lnb = np.any(ln_b != 0.0)
    use_bqk = np.any(qkv_b[:, :2048] != 0.0)
    use_bv = np.any(qkv_b[:, 2048:] != 0.0)
    use_bg = np.any(gate_b != 0.0)
    use_bo = np.any(out_b != 0.0)
    use_bp = np.any(proj_b != 0.0)
    rpb_nz = bool(np.any(rpb != 0.0))

    x0 = emb[input_ids] + pos[:, :S, :]                       # [B, S, D]
    # feature-major per core: [D, S] -> [NC_D, 128, S]
    x0t = np.ascontiguousarray(x0.transpose(0, 2, 1)).reshape(B, NC_D, 128, S)

    # lhsT tiles [K=fi(128), M=fo(128)] packed 8-fi-wide: [l, fo, 128p, 8c*128j]
    def lhs_tiles(w, nfo):  # w: [L, D, nfo*128]
        r = w.reshape(L, NC_D, 128, nfo, 128).transpose(0, 3, 2, 1, 4)
        return np.ascontiguousarray(r.reshape(L, nfo, 128, NC_D * 128))

    wqk = lhs_tiles(qkv_w[:, :, :2048], 16)                   # [6,16,128,1024]
    wg = lhs_tiles(gate_w, NC_D)                              # [6,8,128,1024]
    wo = lhs_tiles(out_w, NC_D)                               # [6,8,128,1024]
    # v-section with ln_g folded into rows (xn = affine(x) * ln_g commutes)
    wv_full = qkv_w[:, :, 2048:]
    if use_lng:
        wv_full = wv_full * ln_g[:, :, None]
    wvs = wv_full.sum(axis=1).reshape(L, 2, 1, 512)           # column sums
    # v-section as moving tiles [K=fi(128), N=fo(512)]: [l, foB*8+fi, 128, 512]
    wv = wv_full.reshape(L, NC_D, 128, 2, 512).transpose(0, 3, 1, 2, 4)
    wv = np.ascontiguousarray(wv.reshape(L, 16, 128, 512))
    wp = proj_w.reshape(NC_D, 128, NC_V, 128).transpose(2, 1, 0, 3)
    wp = np.ascontiguousarray(wp.reshape(NC_V, 128, NC_D * 128))

    tri = np.triu(np.ones((128, 128), f))
    onec = np.ones((128, 1), f)
    oner = np.ones((1, 128), f)

    shared = {
        "wqk": wqk.astype(dt_np), "wv": wv.astype(dt_np), "wg": wg.astype(dt_np),
        "wo": wo.astype(dt_np), "wp": wp.astype(dt_np), "wvs": wvs.astype(dt_np),
        "tri": tri.astype(dt_np), "onec": onec.astype(dt_np),
        "oner": oner.astype(dt_np),
    }
    if use_lng:
        shared["lng"] = np.ascontiguousarray(ln_g.reshape(L, NC_D, 128))
    if use_lnb:
        shared["lnb"] = np.ascontiguousarray(ln_b.reshape(L, NC_D, 128))
        shared["wvb"] = np.ascontiguousarray(
            np.einsum("lf,lfo->lo", ln_b, wv_full).reshape(L, 2, 1, 512)
        ).astype(dt_np)
    if use_bqk:
        shared["bqk"] = np.ascontiguousarray(qkv_b[:, :2048].reshape(L, 16, 128))
    if use_bv:
        shared["bv"] = np.ascontiguousarray(
            qkv_b[:, 2048:].reshape(L, 2, 1, 512)
        ).astype(dt_np)
    if use_bg:
        shared["bg"] = np.ascontiguousarray(gate_b.reshape(L, NC_D, 128))
    if use_bo:
        shared["bo"] = np.ascontiguousarray(out_b.reshape(L, NC_D, 128))
    if use_bp:
        shared["bp"] = np.ascontiguousarray(proj_b.reshape(NC_V, 128))
    if rpb_nz:
        shared["rpb"] = np.ascontiguousarray(rpb.reshape(1, L * H)).astype(dt_np)

    cfg = (mm_mode, use_lng, use_lnb, use_bqk, use_bv, use_bg, use_bo, use_bp, rpb_nz)
    in_maps = []
    for b in range(B):
        m = dict(shared)
        m["x0t"] = np.ascontiguousarray(x0t[b]).astype(dt_np)
        in_maps.append(m)
    return cfg, in_maps


def run(inputs, mm_mode="bf16", trace=False):
    cfg, in_maps = _marshal(inputs, mm_mode)
    nc = _get_program(cfg)
    res = run_bass_kernel_spmd(nc, in_maps, core_ids=list(range(N_CORES)), trace=trace)
    out = np.empty((B, S, V), np.float32)
    for b in range(B):
        lt = np.asarray(res.results[b]["logits_t"], np.float32).reshape(V, S)
        out[b] = lt.T
    return out, res


def kernel(**inputs) -> np.ndarray:
    out, _ = run(inputs, mm_mode="bf16", trace=False)
    return out


# revision 37
# speedup vs baseline: 1.0110x; 1.0110x over previous
"""HSTU-style dense transformer for sequence modeling on 8 Trainium2 NeuronCores.

Sharding: data-parallel over batch (B=8 -> 1 sequence per core). All weights
replicated. Activations are kept feature-major [D=partitions, T=free] on chip;
GEMMs consume bf16 weights as stationary lhsT tiles (fast weight load) and
bf16 activations as the moving operand; attention computes transposed scores
[kt, qt] so the sigmoid/causal-trim/AV chain needs no on-device transposes.

Per-layer critical path: the V projection consumes RAW x (LayerNorm's
per-token affine commutes with the GEMM): v = (x@Wv)*rs[t] - mu[t]*rs[t]*
colsum(Wv), applied via a K=1 rank-1 finisher matmul plus a per-partition
scale at PSUM drain. This keeps the PE busy through the serial LN row-math
chain (which otherwise idles the PE long enough for the HAM clock gate to
drop it to 1.2 GHz).

Host side only marshals: embedding gather + positional add, weight pre-tiling
into DMA-contiguous bf16 layouts, and the final [V,T] -> [S,V] untranspose.
"""

import sys

sys.path.insert(0, "/opt/trn_rl_repo")

import numpy as np

import concourse.bass as bass  # noqa: F401  (keeps bass registered before bacc)
import concourse.tile as tile
from concourse import bacc, mybir
from concourse.bass import ts
from concourse.bass_utils import run_bass_kernel_spmd

B, S, D, H, L, V = 8, 512, 1024, 16, 6, 32000
DH = D // H
LN_EPS = 1e-5
N_CORES = 8
NC_D = D // 128      # 8 feature chunks
NC_T = S // 128      # 4 token chunks
NC_V = V // 128      # 250 vocab chunks
NP = 8               # head pairs

F32 = mybir.dt.float32
BF16 = mybir.dt.bfloat16
AF = mybir.ActivationFunctionType
OP = mybir.AluOpType

_prog_cache = {}


def _build(cfg):
    """Build + compile the SPMD per-core program. cfg is a hashable tuple."""
    (mm_mode, use_lng, use_lnb, use_bqk, use_bv, use_bg, use_bo, use_bp, rpb_nz) = cfg
    assert mm_mode == "bf16"
    DT = BF16

    nc = bacc.Bacc("TRN2", target_bir_lowering=False, debug=False)

    x0_d = nc.dram_tensor("x0t", [NC_D, 128, S], DT, kind="ExternalInput").ap()
    wqk_d = nc.dram_tensor("wqk", [L, 16, 128, 1024], DT, kind="ExternalInput").ap()
    wv_d = nc.dram_tensor("wv", [L, 16, 128, 512], DT, kind="ExternalInput").ap()
    wvs_d = nc.dram_tensor("wvs", [L, 2, 1, 512], DT, kind="ExternalInput").ap()
    wvb_d = nc.dram_tensor("wvb", [L, 2, 1, 512], DT, kind="ExternalInput").ap() if use_lnb else None
    wg_d = nc.dram_tensor("wg", [L, NC_D, 128, 1024], DT, kind="ExternalInput").ap()
    wo_d = nc.dram_tensor("wo", [L, NC_D, 128, 1024], DT, kind="ExternalInput").ap()
    wp_d = nc.dram_tensor("wp", [NC_V, 128, 1024], DT, kind="ExternalInput").ap()
    tri_d = nc.dram_tensor("tri", [128, 128], DT, kind="ExternalInput").ap()
    onec_d = nc.dram_tensor("onec", [128, 1], DT, kind="ExternalInput").ap()
    oner_d = nc.dram_tensor("oner", [1, 128], DT, kind="ExternalInput").ap()
    lng_d = nc.dram_tensor("lng", [L, NC_D, 128], F32, kind="ExternalInput").ap() if use_lng else None
    lnb_d = nc.dram_tensor("lnb", [L, NC_D, 128], F32, kind="ExternalInput").ap() if use_lnb else None
    bqk_d = nc.dram_tensor("bqk", [L, 16, 128], F32, kind="ExternalInput").ap() if use_bqk else None
    bv_d = nc.dram_tensor("bv", [L, 2, 1, 512], DT, kind="ExternalInput").ap() if use_bv else None
    bg_d = nc.dram_tensor("bg", [L, NC_D, 128], F32, kind="ExternalInput").ap() if use_bg else None
    bo_d = nc.dram_tensor("bo", [L, NC_D, 128], F32, kind="ExternalInput").ap() if use_bo else None
    bp_d = nc.dram_tensor("bp", [NC_V, 128], F32, kind="ExternalInput").ap() if use_bp else None
    rpb_d = nc.dram_tensor("rpb", [1, H * L], DT, kind="ExternalInput").ap() if rpb_nz else None
    out_d = nc.dram_tensor("logits_t", [NC_V, 128, S], DT, kind="ExternalOutput").ap()

    with tile.TileContext(nc) as tc, nc.allow_low_precision(
        reason="bf16 tiles feed the PE; accumulation stays fp32 in PSUM"
    ):
        from contextlib import ExitStack

        with ExitStack() as ctx:
            cp = ctx.enter_context(tc.tile_pool(name="consts", bufs=1))
            xp = ctx.enter_context(tc.tile_pool(name="x", bufs=2))
            xnp = ctx.enter_context(tc.tile_pool(name="xn", bufs=1))
            up = ctx.enter_context(tc.tile_pool(name="u", bufs=1))
            vp = ctx.enter_context(tc.tile_pool(name="v", bufs=1))
            gp = ctx.enter_context(tc.tile_pool(name="g", bufs=1))
            qkp = ctx.enter_context(tc.tile_pool(name="qk", bufs=3))
            tmp = ctx.enter_context(tc.tile_pool(name="tmp", bufs=3))
            rows = ctx.enter_context(tc.tile_pool(name="rows", bufs=6))
            bp_pool = ctx.enter_context(tc.tile_pool(name="bcast", bufs=4))
            atp = ctx.enter_context(tc.tile_pool(name="at", bufs=6))
            wbp = ctx.enter_context(tc.tile_pool(name="wb", bufs=6))
            wvp = ctx.enter_context(tc.tile_pool(name="wvp", bufs=1))
            op_pool = ctx.enter_context(tc.tile_pool(name="out", bufs=4))
            prm = ctx.enter_context(tc.tile_pool(name="prm", bufs=2))
            rcp = ctx.enter_context(tc.tile_pool(name="rc", bufs=2))
            pmm = ctx.enter_context(tc.tile_pool(name="pmm", bufs=2, space="PSUM"))
            pao = ctx.enter_context(tc.tile_pool(name="pao", bufs=2, space="PSUM"))
            psc = ctx.enter_context(tc.tile_pool(name="psc", bufs=3, space="PSUM"))
            pst = ctx.enter_context(tc.tile_pool(name="pst", bufs=1, space="PSUM"))

            mm = nc.tensor.matmul

            onec = cp.tile([128, 1], DT)
            nc.sync.dma_start(onec[:], onec_d[:])
            oner = cp.tile([1, 128], DT)
            nc.sync.dma_start(oner[:], oner_d[:])

            x_cur = xp.tile([128, NC_D * S], DT, tag="x")
            for c in range(NC_D):
                eng = nc.sync if c % 2 == 0 else nc.gpsimd
                eng.dma_start(x_cur[:, ts(c, S)], x0_d[c])

            tri_t = cp.tile([128, 128], DT)
            nc.sync.dma_start(tri_t[:], tri_d[:])
            if rpb_nz:
                rpb_row = cp.tile([1, H * L], DT)
                nc.sync.dma_start(rpb_row[:], rpb_d[:])
                # broadcast to [128, H*L] so column slices give per-partition bias
                prb = psc.tile([128, 512], F32, tag="sc")
                mm(prb[:, : H * L], oner[:], rpb_row[:], start=True, stop=True)
                rpb_t = cp.tile([128, H * L], F32)
                nc.scalar.copy(rpb_t[:], prb[:, : H * L])
            if use_bp:
                bp_t = cp.tile([128, NC_V], F32)
                nc.sync.dma_start(bp_t[:], bp_d.rearrange("v p -> p v"))

            for l in range(L):
                # ---- per-layer params + weight prefetch ----
                wv_t = wvp.tile([128, 16 * 512], DT, tag="wv")
                for i in range(16):
                    nc.gpsimd.dma_start(wv_t[:, ts(i, 512)], wv_d[l, i])
                wvs_t = prm.tile([1, 1024], DT, tag="wvs")
                nc.gpsimd.dma_start(wvs_t[:, 0:512], wvs_d[l, 0])
                nc.gpsimd.dma_start(wvs_t[:, 512:1024], wvs_d[l, 1])
                if use_lng:
                    lng_t = prm.tile([128, NC_D], F32, tag="lng")
                    nc.sync.dma_start(lng_t[:], lng_d[l].rearrange("c p -> p c"))
                if use_lnb:
                    lnb_t = prm.tile([128, NC_D], F32, tag="lnb")
                    nc.sync.dma_start(lnb_t[:], lnb_d[l].rearrange("c p -> p c"))
                if use_bqk:
                    bqk_t = prm.tile([128, 16], F32, tag="bqk")
                    nc.sync.dma_start(bqk_t[:], bqk_d[l].rearrange("c p -> p c"))
                if use_bg:
                    bg_t = prm.tile([128, NC_D], F32, tag="bg")
                    nc.sync.dma_start(bg_t[:], bg_d[l].rearrange("c p -> p c"))
                if use_bo:
                    bo_t = prm.tile([128, NC_D], F32, tag="bo")
                    nc.sync.dma_start(bo_t[:], bo_d[l].rearrange("c p -> p c"))

                # ---- LayerNorm stats (feature-major: reduce over partitions via
                # ones-matmul). Sum lands on psum partition 0, sum-of-squares on
                # partition 32 (32-aligned row groups) -> one PSUM bank total.
                ps = pst.tile([64, S], F32, tag="st")
                for c in range(NC_D):
                    xc = x_cur[:, ts(c, S)]
                    mm(ps[0:1, :], onec[:], xc, start=(c == 0), stop=(c == NC_D - 1))
                    sq = tmp.tile([128, S], DT, tag="tmp")
                    nc.vector.tensor_mul(sq[:], xc, xc)
                    mm(ps[32:33, :], onec[:], sq[:], start=(c == 0), stop=(c == NC_D - 1))

                mu = rows.tile([1, S], F32, tag="row")
                nc.scalar.activation(mu[:], ps[0:1, :], AF.Copy, bias=0.0, scale=1.0 / D)
                nmu16 = rows.tile([1, S], DT, tag="row16")
                nc.scalar.activation(nmu16[:], mu[:], AF.Copy, bias=0.0, scale=-1.0)
                musq = rows.tile([1, S], F32, tag="row")
                nc.vector.tensor_mul(musq[:], mu[:], mu[:])
                var = rows.tile([1, S], F32, tag="row")
                nc.vector.scalar_tensor_tensor(
                    var[:], ps[32:33, :], 1.0 / D, musq[:], op0=OP.mult, op1=OP.subtract
                )
                vare = rows.tile([1, S], F32, tag="row")
                nc.vector.tensor_scalar_add(vare[:], var[:], LN_EPS)
                sd = rows.tile([1, S], F32, tag="row")
                nc.scalar.activation(sd[:], vare[:], AF.Sqrt, bias=0.0, scale=1.0)
                rs = rows.tile([1, S], F32, tag="row")
                nc.vector.reciprocal_approx_fast(rs[:], sd[:])
                msr = rows.tile([1, S], F32, tag="row")
                nc.vector.tensor_mul(msr[:], mu[:], rs[:])

                # rs transposed to column form [128 tok, NC_T] for V-psum drains
                rs_col = rcp.tile([128, NC_T], F32, tag="rsc")
                for c in range(NC_T):
                    nc.sync.dma_start(rs_col[:, c : c + 1], rs[0:1, ts(c, 128)])

                # hi/lo split bf16 broadcasts of rs and mu*rs (fp32 accuracy)
                rs_h = rows.tile([1, S], DT, tag="row16")
                nc.scalar.activation(rs_h[:], rs[:], AF.Copy, bias=0.0, scale=1.0)
                rs_l = rows.tile([1, S], DT, tag="row16")
                nc.vector.tensor_sub(rs_l[:], rs[:], rs_h[:])
                ms_h = rows.tile([1, S], DT, tag="row16")
                nc.scalar.activation(ms_h[:], msr[:], AF.Copy, bias=0.0, scale=1.0)
                ms_l = rows.tile([1, S], DT, tag="row16")
                nc.vector.tensor_sub(ms_l[:], msr[:], ms_h[:])
                pb1 = psc.tile([128, S], F32, tag="sc")
                mm(pb1[:], oner[:], rs_h[:], start=True, stop=False)
                mm(pb1[:], oner[:], rs_l[:], start=False, stop=True)
                pb2 = psc.tile([128, S], F32, tag="sc")
                mm(pb2[:], oner[:], ms_h[:], start=True, stop=False)
                mm(pb2[:], oner[:], ms_l[:], start=False, stop=True)
                rs_b = bp_pool.tile([128, S], F32, tag="bb")
                nc.scalar.copy(rs_b[:], pb1[:])
                ms_b = bp_pool.tile([128, S], F32, tag="bb")
                nc.scalar.copy(ms_b[:], pb2[:])

                # ---- V projection from RAW x, token-major [t, fo].
                # v = ((x - mu) * rs) @ Wv  ==  ((x@Wv) - mu (x) colsum(Wv)) * rs[t]
                # (ln_g is folded into Wv rows host-side; the ln_b/qkv_b terms
                # are token-constant rows, broadcast and added after the rs
                # scaling below.)
                post_add = None
                if use_bv or use_lnb:
                    post_add = []
                    for foB in range(2):
                        pvb = psc.tile([128, 512], F32, tag="sc")
                        first = True
                        if use_bv:
                            bvrow = rows.tile([1, 512], DT, tag="bvr")
                            nc.sync.dma_start(bvrow[:], bv_d[l, foB])
                            mm(pvb[:], oner[:], bvrow[:], start=True, stop=(not use_lnb))
                            first = False
                        if use_lnb:
                            lbrow = rows.tile([1, 512], DT, tag="bvr")
                            nc.sync.dma_start(lbrow[:], wvb_d[l, foB])
                            mm(pvb[:], oner[:], lbrow[:], start=first, stop=True)
                        vbt = bp_pool.tile([128, 512], F32, tag="vb")
                        nc.scalar.copy(vbt[:], pvb[:])
                        post_add.append(vbt)
                v = vp.tile([128, NC_T * 1024], DT, tag="v")
                for foB in range(2):
                    for tc_ in range(NC_T):
                        pv = pmm.tile([128, 512], F32, tag="mm")
                        for fi in range(NC_D):
                            mm(
                                pv[:],
                                x_cur[:, fi * S + tc_ * 128 : fi * S + tc_ * 128 + 128],
                                wv_t[:, ts(foB * 8 + fi, 512)],
                                start=(fi == 0),
                                stop=False,
                            )
                        mm(
                            pv[:],
                            nmu16[0:1, ts(tc_, 128)],
                            wvs_t[0:1, ts(foB, 512)],
                            start=False,
                            stop=True,
                        )
                        dst = v[:, tc_ * 1024 + foB * 512 : tc_ * 1024 + foB * 512 + 512]
                        if post_add is not None:
                            t4 = tmp.tile([128, 512], F32, tag="tmp")
                            nc.vector.tensor_scalar_mul(t4[:], pv[:], rs_col[:, tc_ : tc_ + 1])
                            nc.vector.tensor_add(dst, t4[:], post_add[foB][:])
                        else:
                            nc.vector.tensor_scalar_mul(dst, pv[:], rs_col[:, tc_ : tc_ + 1])

                # ---- xn = (x - mu) * rs, feature-major bf16 ----
                xn = xnp.tile([128, NC_D * S], DT, tag="xn")
                for c in range(NC_D):
                    t = tmp.tile([128, S], F32, tag="tmp")
                    nc.vector.tensor_mul(t[:], x_cur[:, ts(c, S)], rs_b[:])
                    if use_lng or use_lnb:
                        t2 = tmp.tile([128, S], F32, tag="tmp")
                        nc.vector.tensor_sub(t2[:], t[:], ms_b[:])
                        nc.scalar.activation(
                            xn[:, ts(c, S)], t2[:], AF.Identity,
                            bias=(lnb_t[:, c : c + 1] if use_lnb else 0.0),
                            scale=(lng_t[:, c : c + 1] if use_lng else 1.0),
                        )
                    else:
                        nc.vector.tensor_sub(xn[:, ts(c, S)], t[:], ms_b[:])

                # ---- gate: u = silu(xn @ gate_w + bg) (feature-major) ----
                u = up.tile([128, NC_D * S], DT, tag="u")
                for f in range(NC_D):
                    wt = wbp.tile([128, 1024], DT, tag="wb")
                    nc.sync.dma_start(wt[:], wg_d[l, f])
                    pu = pmm.tile([128, S], F32, tag="mm")
                    for fi in range(NC_D):
                        mm(pu[:], wt[:, ts(fi, 128)], xn[:, ts(fi, S)],
                           start=(fi == 0), stop=(fi == NC_D - 1))
                    nc.scalar.activation(
                        u[:, ts(f, S)], pu[:], AF.Silu,
                        bias=(bg_t[:, f : f + 1] if use_bg else 0.0), scale=1.0,
                    )

                # ---- attention, one head pair (= one q/k feature chunk) at a time ----
                g = gp.tile([128, NC_D * S], DT, tag="g")
                for p_ in range(NP):
                    qc = qkp.tile([128, S], DT, tag="qc")
                    kc = qkp.tile([128, S], DT, tag="kc")
                    for (dst, fidx) in ((qc, p_), (kc, 8 + p_)):
                        wt = wbp.tile([128, 1024], DT, tag="wb")
                        nc.sync.dma_start(wt[:], wqk_d[l, fidx])
                        pq = pmm.tile([128, S], F32, tag="mm")
                        for fi in range(NC_D):
                            mm(pq[:], wt[:, ts(fi, 128)], xn[:, ts(fi, S)],
                               start=(fi == 0), stop=(fi == NC_D - 1))
                        if use_bqk:
                            nc.scalar.activation(
                                dst[:], pq[:], AF.Identity,
                                bias=bqk_t[:, fidx : fidx + 1], scale=1.0,
                            )
                        else:
                            nc.vector.tensor_copy(dst[:], pq[:])

                    ao = pao.tile([128, S], F32, tag="ao")
                    for c in range(NC_T):
                        n = S - 128 * c
                        for (o, hh) in ((0, 2 * p_), (64, 2 * p_ + 1)):
                            sc = psc.tile([128, S], F32, tag="sc")
                            mm(sc[:, :n], kc[o : o + 64, ts(c, 128)],
                               qc[o : o + 64, c * 128 : S], start=True, stop=True)
                            sig_bias = rpb_t[:, l * H + hh : l * H + hh + 1] if rpb_nz else 0.0
                            att = atp.tile([128, S], DT, tag="at")
                            nc.scalar.activation(att[:, 0:n], sc[:, 0:n], AF.Sigmoid,
                                                 bias=sig_bias, scale=DH**-0.5)
                            nc.vector.tensor_mul(att[:, 0:128], att[:, 0:128], tri_t[:])
                            mm(
                                ao[o : o + 64, c * 128 : S],
                                v[:, c * 1024 + p_ * 128 + o : c * 1024 + p_ * 128 + o + 64],
                                att[:, 0:n],
                                start=(c == 0),
                                stop=(c == NC_T - 1),
                            )
                    nc.vector.tensor_mul(g[:, ts(p_, S)], ao[:], u[:, ts(p_, S)])

                # ---- out projection + residual ----
                x_new = xp.tile([128, NC_D * S], DT, tag="x")
                for f in range(NC_D):
                    wt = wbp.tile([128, 1024], DT, tag="wb")
                    nc.sync.dma_start(wt[:], wo_d[l, f])
                    pd = pmm.tile([128, S], F32, tag="mm")
                    for fi in range(NC_D):
                        mm(pd[:], wt[:, ts(fi, 128)], g[:, ts(fi, S)],
                           start=(fi == 0), stop=(fi == NC_D - 1))
                    if use_bo:
                        t3 = tmp.tile([128, S], F32, tag="tmp")
                        nc.scalar.activation(t3[:], pd[:], AF.Identity,
                                             bias=bo_t[:, f : f + 1], scale=1.0)
                        nc.vector.tensor_add(x_new[:, ts(f, S)], t3[:], x_cur[:, ts(f, S)])
                    else:
                        nc.vector.tensor_add(x_new[:, ts(f, S)], pd[:], x_cur[:, ts(f, S)])
                x_cur = x_new

            # ---- logits: [V, T] feature(vocab)-major ----
            for vo in range(NC_V):
                wt = wbp.tile([128, 1024], DT, tag="wb")
                nc.sync.dma_start(wt[:], wp_d[vo])
                pl = pmm.tile([128, S], F32, tag="mm")
                for fi in range(NC_D):
                    mm(pl[:], wt[:, ts(fi, 128)], x_cur[:, ts(fi, S)],
                       start=(fi == 0), stop=(fi == NC_D - 1))
                ot = op_pool.tile([128, S], DT, tag="o")
                if use_bp:
                    nc.scalar.activation(ot[:], pl[:], AF.Identity,
                                         bias=bp_t[:, vo : vo + 1], scale=1.0)
                elif vo % 2 == 0:
                    nc.scalar.copy(ot[:], pl[:])
                else:
                    nc.vector.tensor_copy(ot[:], pl[:])
                nc.gpsimd.dma_start(out_d[vo], ot[:])

    nc.compile()
    return nc


def _get_program(cfg):
    nc = _prog_cache.get(cfg)
    if nc is None:
        nc = _build(cfg)
        _prog_cache[cfg] = nc
    return nc


def _marshal(inputs, mm_mode="bf16"):
    """Host-side input marshalling into DMA-friendly layouts."""
    import ml_dtypes

    dt_np = ml_dtypes.bfloat16
    f = np.float32
    input_ids = np.asarray(inputs["input_ids"])
    emb = np.asarray(inputs["embedding"], f)
    pos = np.asarray(inputs["pos_encoding"], f)
    qkv_w = np.asarray(inputs["qkv_w"], f)
    gate_w = np.asarray(inputs["gate_w"], f)
    out_w = np.asarray(inputs["out_w"], f)
    proj_w = np.asarray(inputs["proj_w"], f)

    qkv_b = np.asarray(inputs["qkv_b"], f)
    gate_b = np.asarray(inputs["gate_b"], f)
    out_b = np.asarray(inputs["out_b"], f)
    proj_b = np.asarray(inputs["proj_b"], f)
    ln_g = np.asarray(inputs["ln_g"], f)
    ln_b = np.asarray(inputs["ln_b"], f)
    rpb = np.asarray(inputs["rel_pos_bias"], f)

    use_lng = not np.all(ln_g == 1.0)
    use_lnb = np.any(ln_b != 0.0)
    use_bqk = np.any(qkv_b[:, :2048] != 0.0)
    use_bv = np.any(qkv_b[:, 2048:] != 0.0)
    use_bg = np.any(gate_b != 0.0)
    use_bo = np.any(out_b != 0.0)
    use_bp = np.any(proj_b != 0.0)
    rpb_nz = bool(np.any(rpb != 0.0))

    x0 = emb[input_ids] + pos[:, :S, :]                       # [B, S, D]
    # feature-major per core: [D, S] -> [NC_D, 128, S]
    x0t = np.ascontiguousarray(x0.transpose(0, 2, 1)).reshape(B, NC_D, 128, S)

    # lhsT tiles [K=fi(128), M=fo(128)] packed 8-fi-wide: [l, fo, 128p, 8c*128j]
    def lhs_tiles(w, nfo):  # w: [L, D, nfo*128]
        r = w.reshape(L, NC_D, 128, nfo, 128).transpose(0, 3, 2, 1, 4)
        return np.ascontiguousarray(r.reshape(L, nfo, 128, NC_D * 128))

    wqk = lhs_tiles(qkv_w[:, :, :2048], 16)                   # [6,16,128,1024]
    wg = lhs_tiles(gate_w, NC_D)                              # [6,8,128,1024]
    wo = lhs_tiles(out_w, NC_D)                               # [6,8,128,1024]
    # v-section with ln_g folded into rows (xn = affine(x) * ln_g commutes)
    wv_full = qkv_w[:, :, 2048:]
    if use_lng:
        wv_full = wv_full * ln_g[:, :, None]
    wvs = wv_full.sum(axis=1).reshape(L, 2, 1, 512)           # column sums
    # v-section as moving tiles [K=fi(128), N=fo(512)]: [l, foB*8+fi, 128, 512]
    wv = wv_full.reshape(L, NC_D, 128, 2, 512).transpose(0, 3, 1, 2, 4)
    wv = np.ascontiguousarray(wv.reshape(L, 16, 128, 512))
    wp = proj_w.reshape(NC_D, 128, NC_V, 128).transpose(2, 1, 0, 3)
    wp = np.ascontiguousarray(wp.reshape(NC_V, 128, NC_D * 128))

    tri = np.triu(np.ones((128, 128), f))
    onec = np.ones((128, 1), f)
    oner = np.ones((1, 128), f)

    shared = {
        "wqk": wqk.astype(dt_np), "wv": wv.astype(dt_np), "wg": wg.astype(dt_np),
        "wo": wo.astype(dt_np), "wp": wp.astype(dt_np), "wvs": wvs.astype(dt_np),
        "tri": tri.astype(dt_np), "onec": onec.astype(dt_np),
        "oner": oner.astype(dt_np),
    }
    if use_lng:
        shared["lng"] = np.ascontiguousarray(ln_g.reshape(L, NC_D, 128))
    if use_lnb:
        shared["lnb"] = np.ascontiguousarray(ln_b.reshape(L, NC_D, 128))
        shared["wvb"] = np.ascontiguousarray(
            np.einsum("lf,lfo->lo", ln_b, wv_full).reshape(L, 2, 1, 512)
        ).astype(dt_np)
    if use_bqk:
        shared["bqk"] = np.ascontiguousarray(qkv_b[:, :2048].reshape(L, 16, 128))
    if use_bv:
        shared["bv"] = np.ascontiguousarray(
            qkv_b[:, 2048:].reshape(L, 2, 1, 512)
        ).astype(dt_np)
    if use_bg:
        shared["bg"] = np.ascontiguousarray(gate_b.reshape(L, NC_D, 128))
    if use_bo:
        shared["bo"] = np.ascontiguousarray(out_b.reshape(L, NC_D, 128))
    if use_bp:
        shared["bp"] = np.ascontiguousarray(proj_b.reshape(NC_V, 128))
    if rpb_nz:
        shared["rpb"] = np.ascontiguousarray(rpb.reshape(1, L * H)).astype(dt_np)

    cfg = (mm_mode, use_lng, use_lnb, use_bqk, use_bv, use_bg, use_bo, use_bp, rpb_nz)
    in_maps = []
    for b in range(B):
        m = dict(shared)
        m["x0t"] = np.ascontiguousarray(x0t[b]).astype(dt_np)
        in_maps.append(m)
    return cfg, in_maps


def run(inputs, mm_mode="bf16", trace=False):
    cfg, in_maps = _marshal(inputs, mm_mode)
    nc = _get_program(cfg)
    res = run_bass_kernel_spmd(nc, in_maps, core_ids=list(range(N_CORES)), trace=trace)
    out = np.empty((B, S, V), np.float32)
    for b in range(B):
        lt = np.asarray(res.results[b]["logits_t"], np.float32).reshape(V, S)
        out[b] = lt.T
    return out, res


def kernel(**inputs) -> np.ndarray:
    out, _ = run(inputs, mm_mode="bf16", trace=False)
    return out
